# revision 29
# baseline (speedup 1.0000x reference)
"""Trainium2 Bass kernel for GuidedImplicitPointSampler KNN (top-8 + occupancy mask).

Strategy (pruned, exact):
  - Host groups the N=32768 queries into 256 spatial tiles of 128 (k-d median
    splits) and, per tile, builds a provably sufficient candidate subset of the
    M=16384 targets from grid cell COUNTS only (no host distance math):
      * ub8(q): walk cell offsets sorted by worst-case point-to-point distance
        until >= 9 targets are guaranteed; d8(q) <= ub8(q).  Two-level grid
        (coarse 0.30 everywhere, fine 0.06 refine in dense regions).
      * tile candidates: every target within R_t = max_q ub8(q) of the tile's
        bbox (cylinder-trimmed cell ranges; superset by construction).
    The device then computes exact distances + top-8 over the candidates, so
    the result equals brute force (candidates contain each query's true 8-NN
    and its nearest target, which also decides the 0.25 occupancy mask).
  - Tiles are dealt to 8 cores x 32 slots (sorted by size, groups of 8) so the
    SPMD program sees identical slot capacities; blocks are sentinel-padded.
  - Per slot: s[n,m] = 2q.k - |k|^2 on the PE as one K=11 fp16 hi/lo matmul
    (error ~2^-22), chunks of <=512 columns packed 4-wide into PE row groups
    0/32/64/96; top-8 via hardware MAX8 straight out of PSUM.
  - Epilogue: d = sqrt(max(q2 - s, 0)), zero rows whose nearest dist <= 0.25;
    host scatters rows back to the original query order.
"""

import numpy as np

N = 32768
M = 16384
KNN = 8
OCC_RADIUS = 0.25
N_CORES = 8
TILE = 128
NTILES = N // TILE            # 256
RT = NTILES // N_CORES        # 32 slots per core
CHUNK = 512                   # matmul moving free dim (one PSUM bank)
GROUP = 1024                  # target cols per big-slot PSUM tile (2 chunks)
BATCH_MAX = 4096              # small-slot DMA batch columns
KDIM = 11
KSAFE = 9
SENTINEL = 60.0

_CACHE = {}


# ---------------------------------------------------------------------------
# Host-side pruning plan (grid counting only, no host distance computations)
# ---------------------------------------------------------------------------

def _cell_counts(pts, lo, h, n):
    ci = np.clip(((pts - lo) / h).astype(np.int64), 0, n - 1)
    cnt = np.zeros((n, n, n), np.int32)
    np.add.at(cnt, (ci[:, 0], ci[:, 1], ci[:, 2]), 1)
    return ci, cnt


def _sorted_offsets(max_cells):
    r = np.arange(-max_cells, max_cells + 1)
    X, Y, Z = np.meshgrid(r, r, r, indexing="ij")
    off = np.stack([X.ravel(), Y.ravel(), Z.ravel()], 1)
    wd = np.sqrt(((np.abs(off) + 1) ** 2).sum(1).astype(np.float64))
    o = np.argsort(wd, kind="stable")
    return off[o], wd[o]


def _walk_ub(cells, cnt, n, offsets, wdist, h, ksafe, chunk=512):
    """Per cell row: smallest wdist*h whose offset-prefix covers >= ksafe targets."""
    U = len(cells)
    ub = np.full(U, np.inf)
    acc = np.zeros(U, np.int64)
    alive = np.arange(U)
    for s in range(0, len(offsets), chunk):
        if len(alive) == 0:
            break
        offs = offsets[s:s + chunk]
        cc = cells[alive][:, None, :] + offs[None, :, :]
        ok = ((cc >= 0) & (cc < n)).all(2)
        cc = np.clip(cc, 0, n - 1)
        counts = cnt[cc[..., 0], cc[..., 1], cc[..., 2]] * ok
        ccum = counts.cumsum(1) + acc[alive][:, None]
        crossed = ccum >= ksafe
        hit = crossed.any(1)
        first = np.argmax(crossed, 1)
        hit_rows = alive[hit]
        ub[hit_rows] = wdist[s + first[hit]] * h
        acc[alive] = ccum[:, -1]
        alive = alive[~hit]
    return ub


def _kd_tiles(q, leaf=TILE):
    out = []

    def rec(ids):
        if len(ids) <= leaf:
            out.append(ids)
            return
        pts = q[ids]
        d = np.argmax(pts.max(0) - pts.min(0))
        half = ((len(ids) // 2) // leaf) * leaf
        o = np.argsort(pts[:, d], kind="stable")
        rec(ids[o[:half]])
        rec(ids[o[half:]])

    rec(np.arange(len(q)))
    return np.concatenate(out)


def _build_plan(q, k, hc=0.30, hm=0.15, hf=0.05, hg=0.08,
                refine_thr_m=3.0, refine_thr=0.9, safety=1.01):
    lo = float(min(q.min(), k.min())) - 1e-4
    hi = float(max(q.max(), k.max())) + 1e-4

    # per-query upper bound on the 8-NN distance: coarse everywhere, then
    # medium / fine refinement where the bound is already small
    nc_ = int(np.ceil((hi - lo) / hc))
    qic = np.clip(((q - lo) / hc).astype(np.int64), 0, nc_ - 1)
    _, cntc = _cell_counts(k, lo, hc, nc_)
    cells_u, inv = np.unique(qic, axis=0, return_inverse=True)
    offc, wdc = _sorted_offsets(nc_)
    ub = _walk_ub(cells_u, cntc, nc_, offc, wdc, hc, KSAFE)[inv]
    assert np.isfinite(ub).all()

    for h_r, thr in ((hm, refine_thr_m), (hf, refine_thr)):
        n_r = int(np.ceil((hi - lo) / h_r))
        qir = np.clip(((q - lo) / h_r).astype(np.int64), 0, n_r - 1)
        _, cnt_r = _cell_counts(k, lo, h_r, n_r)
        ref = ub <= thr
        if not ref.any():
            continue
        cells_r, invr = np.unique(qir[ref], axis=0, return_inverse=True)
        off_r, wd_r = _sorted_offsets(int(np.ceil(thr / h_r)) + 1)
        ubr = _walk_ub(cells_r, cnt_r, n_r, off_r, wd_r, h_r, KSAFE)[invr]
        idx = np.nonzero(ref)[0]
        better = ubr < ub[ref]
        ub[idx[better]] = ubr[better]
    ub *= safety

    perm = _kd_tiles(q)

    # gather CSR over the gather grid
    ng = int(np.ceil((hi - lo) / hg))
    kig = np.clip(((k - lo) / hg).astype(np.int64), 0, ng - 1)
    kcell = (kig[:, 0] * ng + kig[:, 1]) * ng + kig[:, 2]
    korder = np.argsort(kcell, kind="stable")
    kcs = kcell[korder]
    starts = np.searchsorted(kcs, np.arange(ng * ng * ng))
    ends = np.searchsorted(kcs, np.arange(ng * ng * ng), side="right")

    def gather(qs):
        R = float(ub[qs].max())
        R2 = R * R
        blo, bhi = q[qs].min(0), q[qs].max(0)
        a = np.maximum(((blo - R - lo) / hg).astype(np.int64), 0)
        b = np.minimum(((bhi + R - lo) / hg).astype(np.int64), ng - 1)
        parts = []
        for ix in range(a[0], b[0] + 1):
            cx0, cx1 = lo + ix * hg, lo + (ix + 1) * hg
            dx = max(blo[0] - cx1, cx0 - bhi[0], 0.0)
            if dx * dx > R2:
                continue
            for iy in range(a[1], b[1] + 1):
                cy0, cy1 = lo + iy * hg, lo + (iy + 1) * hg
                dy = max(blo[1] - cy1, cy0 - bhi[1], 0.0)
                dxy2 = dx * dx + dy * dy
                if dxy2 > R2:
                    continue
                zh = float(np.sqrt(R2 - dxy2))
                z0 = max(int((blo[2] - zh - lo) / hg), 0)
                z1 = min(int((bhi[2] + zh - lo) / hg), ng - 1)
                base = (ix * ng + iy) * ng
                s, e = starts[base + z0], ends[base + z1]
                if e > s:
                    parts.append(korder[s:e])
        return (np.concatenate(parts) if parts else np.empty(0, np.int64))

    # adaptive tiles: start from 128-query kd leaves; split a tile while the
    # two halves' candidate sets are sufficiently smaller than the parent's
    tiles = []

    def consider(qs, cand, depth):
        if len(cand) > 1024 and len(qs) >= 64 and depth < 4:
            pts = q[qs]
            dim = np.argmax(pts.max(0) - pts.min(0))
            o = np.argsort(pts[:, dim], kind="stable")
            half = len(qs) // 2
            qa, qb = qs[o[:half]], qs[o[half:]]
            ca, cb = gather(qa), gather(qb)
            if len(ca) + len(cb) + 384 < len(cand):
                consider(qa, ca, depth + 1)
                consider(qb, cb, depth + 1)
                return
        tiles.append((qs, cand))

    for t in range(NTILES):
        qs = perm[t * TILE:(t + 1) * TILE]
        consider(qs, gather(qs), 0)

    # pad tile count to a multiple of N_CORES with empty dummy tiles
    while len(tiles) % N_CORES != 0:
        tiles.append((np.empty(0, np.int64), np.empty(0, np.int64)))

    # deal tiles to cores/slots: sort by size asc (small slots first for fast
    # pipeline start), slot i <- tiles [8i, 8i+8)
    sizes = np.array([len(c) for _, c in tiles])
    order = np.argsort(sizes, kind="stable")
    rt = len(tiles) // N_CORES
    tile_of = order.reshape(rt, N_CORES)            # [slot, core]
    caps = np.empty(rt, np.int64)
    for i in range(rt):
        caps[i] = max(int(np.ceil(sizes[tile_of[i]].max() / 64.0)) * 64, 64)
    return tiles, tile_of, caps


def _f16_split(x):
    h = x.astype(np.float16)
    l = (x - h.astype(np.float32)).astype(np.float16)
    return h, l


def _rhs_block(kpts):
    """[11, C] fp16 block: rows = [(2k)h x3, (2k)h x3, (2k)l x3, -|k|2h, -|k|2l]."""
    k2 = (kpts * kpts).sum(1, dtype=np.float32)
    kh, kl = _f16_split(2.0 * kpts.T)
    k2h, k2l = _f16_split(k2)
    blk = np.empty((KDIM, len(kpts)), np.float16)
    blk[0:3] = kh
    blk[3:6] = kh
    blk[6:9] = kl
    blk[9] = -k2h
    blk[10] = -k2l
    return blk


def _layout(caps):
    """Shared DRAM layout plan for rhs_all.

    Small slots (cap <= 512) are grouped into batches of <= BATCH_MAX columns;
    within a batch, slot j rides PE row-group 32*(j%4) and the DRAM block
    orders slots by row-group so each row-group is one contiguous DMA.
    Big slots follow, laid out contiguously per slot.
    Returns (batches, bigs, slot_off, capsum) where batches is a list of
    [(slot, rg, dram_off, sbuf_off)], bigs a list of slot ids.
    """
    rt = len(caps)
    batches = []
    off = 0
    slot_off = [0] * rt
    i = 0
    while i < rt and caps[i] <= 512:
        j = i
        tot = 0
        bmax = 1024 if not batches else BATCH_MAX
        while j < rt and caps[j] <= 512 and tot + caps[j] <= bmax:
            tot += caps[j]
            j += 1
        slots = list(range(i, j))
        rg_of = {s: idx % 4 for idx, s in enumerate(slots)}
        entries = []
        sbuf_off = {}
        # sbuf offsets restart per row-group
        for p in range(4):
            sb = 0
            for s in slots:
                if rg_of[s] == p:
                    sbuf_off[s] = sb
                    sb += caps[s]
        for p in range(4):
            for s in slots:
                if rg_of[s] == p:
                    entries.append((s, p, off, sbuf_off[s]))
                    slot_off[s] = off
                    off += caps[s]
        batches.append(entries)
        i = j
    bigs = list(range(i, rt))
    for s in bigs:
        slot_off[s] = off
        off += caps[s]
    return batches, bigs, slot_off, off


def _prep_pruned(to_filter, target_coords):
    q = np.ascontiguousarray(np.asarray(to_filter, np.float32)[:, :3])
    k = np.ascontiguousarray(np.asarray(target_coords, np.float32)[:, :3])
    tiles, tile_of, caps = _build_plan(q, k)
    rt = len(caps)
    capsum = int(caps.sum())
    _, _, slot_off, lay_total = _layout(tuple(int(x) for x in caps))
    assert lay_total == capsum

    sent = np.full(3, SENTINEL, np.float32)
    in_maps = []
    rows_per_core = []
    for c in range(N_CORES):
        qsel = np.zeros((rt, TILE), np.int64)
        rows = []
        rhs_all = np.empty((KDIM, capsum), np.float16)
        for i in range(rt):
            t = tile_of[i, c]
            qs, cand = tiles[t]
            rows.append(qs)
            if len(qs):
                qsel[i, :len(qs)] = qs
                qsel[i, len(qs):] = qs[0]
            cap = int(caps[i])
            kp = np.empty((cap, 3), np.float32)
            kp[:len(cand)] = k[cand]
            kp[len(cand):] = sent
            rhs_all[:, slot_off[i]:slot_off[i] + cap] = _rhs_block(kp)
        rows_per_core.append(rows)
        qc = q[qsel.ravel()]
        q2 = (qc * qc).sum(1, dtype=np.float32)
        qh, ql = _f16_split(qc.T)
        lhsT = np.empty((KDIM, rt * TILE), np.float16)
        lhsT[0:3] = qh
        lhsT[3:6] = ql
        lhsT[6:9] = qh
        lhsT[9] = 1.0
        lhsT[10] = 1.0
        q2c = q2.reshape(rt, TILE).T                       # [128, rt]
        in_maps.append({
            "lhsT": np.ascontiguousarray(lhsT),
            "rhs_all": np.ascontiguousarray(rhs_all),
            "q2rep": np.ascontiguousarray(np.repeat(q2c, KNN, axis=1)),
        })
    return in_maps, rows_per_core, tuple(int(x) for x in caps)


def _build_pruned(caps):
    key = ("pruned", caps)
    if key in _CACHE:
        return _CACHE[key]
    from concourse import bacc, tile, mybir

    dt = mybir.dt
    capsum = sum(caps)
    rt = len(caps)
    npc = rt * TILE
    nc = bacc.Bacc("TRN2", target_bir_lowering=False, debug=False,
                   num_devices=N_CORES)

    lhsT_d = nc.dram_tensor("lhsT", [KDIM, npc], dt.float16, kind="ExternalInput")
    rhs_d = nc.dram_tensor("rhs_all", [KDIM, capsum], dt.float16,
                           kind="ExternalInput")
    q2_d = nc.dram_tensor("q2rep", [128, rt * KNN], dt.float32,
                          kind="ExternalInput")
    out_d = nc.dram_tensor("out", [128, rt * KNN], dt.float32,
                           kind="ExternalOutput")

    with tile.TileContext(nc) as tc:
        with (
            tc.tile_pool(name="const", bufs=1) as constp,
            tc.tile_pool(name="rhs", bufs=6) as rhsp,
            tc.tile_pool(name="psum", bufs=2, space="PSUM") as psump,
            tc.tile_pool(name="cand", bufs=2) as candp,
            tc.tile_pool(name="fin", bufs=1) as finp,
        ):
            # lhs split: a small first piece lets slot 0's matmul start while
            # the rest of the queries stream in; 4 replicas, one per PE
            # row-group
            lhs_sb = constp.tile([128, npc], dt.float16)
            l0 = min(8 * TILE, npc)
            for p in range(4):
                eng = nc.sync if p % 2 == 0 else nc.scalar
                eng.dma_start(out=lhs_sb[32 * p:32 * p + KDIM, :l0],
                              in_=lhsT_d[:, :l0])

            # q2 rides the SWDGE path so it never queues behind the rhs
            # stream on either HWDGE ring
            q2_sb = constp.tile([128, rt * KNN], dt.float32)
            nc.gpsimd.dma_start(out=q2_sb[:, :], in_=q2_d[:, :])
            s8_all = finp.tile([128, rt * KNN], dt.float32)
            dsq = finp.tile([128, rt * KNN], dt.float32)
            droot = finp.tile([128, rt * KNN], dt.float32)
            good = finp.tile([128, rt], dt.float32)
            res = finp.tile([128, rt * KNN], dt.float32)

            def epilogue(a, b):
                # d = sqrt(max(q2 - s, 0)); zero rows whose min d2 <= OCC^2
                ca, cb = a * KNN, b * KNN
                nc.vector.tensor_sub(dsq[:, ca:cb], q2_sb[:, ca:cb],
                                     s8_all[:, ca:cb])
                nc.vector.tensor_scalar(good[:, a:b], dsq[:, ca:cb:KNN],
                                        OCC_RADIUS * OCC_RADIUS, None,
                                        mybir.AluOpType.is_gt)
                nc.vector.tensor_scalar_max(dsq[:, ca:cb], dsq[:, ca:cb], 0.0)
                nc.scalar.activation(droot[:, ca:cb], dsq[:, ca:cb],
                                     mybir.ActivationFunctionType.Sqrt)
                nc.vector.tensor_tensor(
                    res[:, ca:cb].rearrange("p (t j) -> p t j", j=KNN),
                    droot[:, ca:cb].rearrange("p (t j) -> p t j", j=KNN),
                    good[:, a:b, None].broadcast_to([128, b - a, KNN]),
                    mybir.AluOpType.mult,
                )
                nc.sync.dma_start(out=out_d.ap()[:, ca:cb],
                                  in_=res[:, ca:cb])

            batches, bigs, slot_off, _ = _layout(caps)

            # Small slots (cap <= 512): 4 consecutive slots ride the 4 PE
            # row-groups concurrently; one DMA per row-group per batch.
            nbatch = 0
            lhs_rest_sent = False
            for entries in batches:
                rhs_sb = rhsp.tile([128, BATCH_MAX], dt.float16, tag="rhsb")
                for p in range(4):
                    rg = [e for e in entries if e[1] == p]
                    if not rg:
                        continue
                    d0 = rg[0][2]
                    tot = sum(caps[e[0]] for e in rg)
                    eng = nc.sync if (nbatch + p) % 2 == 0 else nc.scalar
                    nc_eng = eng
                    nc_eng.dma_start(
                        out=rhs_sb[32 * p:32 * p + KDIM, :tot],
                        in_=rhs_d[:, d0:d0 + tot])
                nbatch += 1
                if nbatch == 2 and not lhs_rest_sent and l0 < npc:
                    for p in range(4):
                        eng = nc.sync if p % 2 == 0 else nc.scalar
                        eng.dma_start(out=lhs_sb[32 * p:32 * p + KDIM, l0:],
                                      in_=lhsT_d[:, l0:])
                    lhs_rest_sent = True
                for s, p, d0, sb0 in entries:
                    cap = caps[s]
                    tcol = slice(s * TILE, (s + 1) * TILE)
                    ps = psump.tile([128, CHUNK], dt.float32, tag="pss",
                                    bufs=4)
                    nc.tensor.matmul(
                        out=ps[:, :cap],
                        lhsT=lhs_sb[32 * p:32 * p + KDIM, tcol],
                        rhs=rhs_sb[32 * p:32 * p + KDIM, sb0:sb0 + cap],
                        start=True, stop=True,
                        tile_position=(32 * p, 0),
                    )
                    nc.vector.max(out=s8_all[:, s * KNN:(s + 1) * KNN],
                                  in_=ps[:, :cap])
            if not lhs_rest_sent and l0 < npc:
                for p in range(4):
                    eng = nc.sync if p % 2 == 0 else nc.scalar
                    eng.dma_start(out=lhs_sb[32 * p:32 * p + KDIM, l0:],
                                  in_=lhsT_d[:, l0:])
                lhs_rest_sent = True
            epi_done = bigs[0] if bigs else rt
            epilogue(0, epi_done)

            # Big slots: 1024-col PSUM groups, the 2 chunks of a group ride
            # PE row-groups 0/32, one HWDGE ring per chunk.
            for s in bigs:
                cap = caps[s]
                ngroups = (cap + GROUP - 1) // GROUP
                cands = None
                if ngroups > 1:
                    cands = candp.tile([128, ngroups * KNN], dt.float32,
                                       tag="cands")
                tcol = slice(s * TILE, (s + 1) * TILE)
                for g in range(ngroups):
                    g0 = g * GROUP
                    gw = min(GROUP, cap - g0)
                    widths = [min(CHUNK, gw - j * CHUNK)
                              for j in range((gw + CHUNK - 1) // CHUNK)]
                    rhs_sb = rhsp.tile([64, GROUP], dt.float16, tag="rhs")
                    base = slot_off[s] + g0
                    c0 = 0
                    for j, w in enumerate(widths):
                        p = 32 * (j % 2)
                        eng = nc.sync if j % 2 == 0 else nc.scalar
                        eng.dma_start(out=rhs_sb[p:p + KDIM, :w],
                                      in_=rhs_d[:, base + c0:base + c0 + w])
                        c0 += w
                    ps = psump.tile([128, GROUP], dt.float32, tag="ps",
                                    bufs=2)
                    c0 = 0
                    for j, w in enumerate(widths):
                        p = 32 * (j % 2)
                        nc.tensor.matmul(
                            out=ps[:, c0:c0 + w],
                            lhsT=lhs_sb[p:p + KDIM, tcol],
                            rhs=rhs_sb[p:p + KDIM, :w],
                            start=True, stop=True,
                            tile_position=(p, 0),
                        )
                        c0 += w
                    dst = (s8_all[:, s * KNN:(s + 1) * KNN] if ngroups == 1
                           else cands[:, g * KNN:(g + 1) * KNN])
                    nc.vector.max(out=dst, in_=ps[:, :gw])
                if ngroups > 1:
                    nc.vector.max(out=s8_all[:, s * KNN:(s + 1) * KNN],
                                  in_=cands[:, :])
            if epi_done < rt:
                epilogue(epi_done, rt)

    nc.compile()
    _CACHE[key] = nc
    return nc


def _run(to_filter, target_coords, trace=False):
    from concourse import bass_utils

    in_maps, rows_per_core, caps = _prep_pruned(to_filter, target_coords)
    nc = _build_pruned(caps)
    res = bass_utils.run_bass_kernel_spmd(
        nc, in_maps, core_ids=list(range(N_CORES)), trace=trace,
    )
    rt = len(caps)
    out = np.empty((N, KNN), np.float32)
    for c in range(N_CORES):
        oc = res.results[c]["out"].reshape(128, rt, KNN)
        for i, qs in enumerate(rows_per_core[c]):
            if len(qs):
                out[qs] = oc[:len(qs), i, :]
    return out, res


def kernel(to_filter, target_coords):
    out, _ = _run(to_filter, target_coords)
    return out


# revision 30
# speedup vs baseline: 1.0937x; 1.0937x over previous
"""Trainium2 Bass kernel for GuidedImplicitPointSampler KNN (top-8 + occupancy mask).

Strategy (pruned, exact):
  - Host groups the N=32768 queries into 256 spatial tiles of 128 (k-d median
    splits) and, per tile, builds a provably sufficient candidate subset of the
    M=16384 targets from grid cell COUNTS only (no host distance math):
      * ub8(q): walk cell offsets sorted by worst-case point-to-point distance
        until >= 9 targets are guaranteed; d8(q) <= ub8(q).  Two-level grid
        (coarse 0.30 everywhere, fine 0.06 refine in dense regions).
      * tile candidates: every target within R_t = max_q ub8(q) of the tile's
        bbox (cylinder-trimmed cell ranges; superset by construction).
    The device then computes exact distances + top-8 over the candidates, so
    the result equals brute force (candidates contain each query's true 8-NN
    and its nearest target, which also decides the 0.25 occupancy mask).
  - Tiles are dealt to 8 cores x 32 slots (sorted by size, groups of 8) so the
    SPMD program sees identical slot capacities; blocks are sentinel-padded.
  - Per slot: s[n,m] = 2q.k - |k|^2 on the PE as one K=11 fp16 hi/lo matmul
    (error ~2^-22), chunks of <=512 columns packed 4-wide into PE row groups
    0/32/64/96; top-8 via hardware MAX8 straight out of PSUM.
  - Epilogue: d = sqrt(max(q2 - s, 0)), zero rows whose nearest dist <= 0.25;
    host scatters rows back to the original query order.
"""

import numpy as np

N = 32768
M = 16384
KNN = 8
OCC_RADIUS = 0.25
N_CORES = 8
TILE = 128
NTILES = N // TILE            # 256
RT = NTILES // N_CORES        # 32 slots per core
CHUNK = 512                   # matmul moving free dim (one PSUM bank)
GROUP = 1024                  # target cols per big-slot PSUM tile (2 chunks)
BATCH_MAX = 4096              # small-slot DMA batch columns
KDIM = 11
KSAFE = 9
SENTINEL = 60.0

_CACHE = {}


# ---------------------------------------------------------------------------
# Host-side pruning plan (grid counting only, no host distance computations)
# ---------------------------------------------------------------------------

def _cell_counts(pts, lo, h, n):
    ci = np.clip(((pts - lo) / h).astype(np.int64), 0, n - 1)
    cnt = np.zeros((n, n, n), np.int32)
    np.add.at(cnt, (ci[:, 0], ci[:, 1], ci[:, 2]), 1)
    return ci, cnt


def _sorted_offsets(max_cells):
    r = np.arange(-max_cells, max_cells + 1)
    X, Y, Z = np.meshgrid(r, r, r, indexing="ij")
    off = np.stack([X.ravel(), Y.ravel(), Z.ravel()], 1)
    wd = np.sqrt(((np.abs(off) + 1) ** 2).sum(1).astype(np.float64))
    o = np.argsort(wd, kind="stable")
    return off[o], wd[o]


def _walk_ub(cells, cnt, n, offsets, wdist, h, ksafe, chunk=512):
    """Per cell row: smallest wdist*h whose offset-prefix covers >= ksafe targets."""
    U = len(cells)
    ub = np.full(U, np.inf)
    acc = np.zeros(U, np.int64)
    alive = np.arange(U)
    for s in range(0, len(offsets), chunk):
        if len(alive) == 0:
            break
        offs = offsets[s:s + chunk]
        cc = cells[alive][:, None, :] + offs[None, :, :]
        ok = ((cc >= 0) & (cc < n)).all(2)
        cc = np.clip(cc, 0, n - 1)
        counts = cnt[cc[..., 0], cc[..., 1], cc[..., 2]] * ok
        ccum = counts.cumsum(1) + acc[alive][:, None]
        crossed = ccum >= ksafe
        hit = crossed.any(1)
        first = np.argmax(crossed, 1)
        hit_rows = alive[hit]
        ub[hit_rows] = wdist[s + first[hit]] * h
        acc[alive] = ccum[:, -1]
        alive = alive[~hit]
    return ub


def _kd_tiles(q, leaf=TILE):
    out = []

    def rec(ids):
        if len(ids) <= leaf:
            out.append(ids)
            return
        pts = q[ids]
        d = np.argmax(pts.max(0) - pts.min(0))
        half = ((len(ids) // 2) // leaf) * leaf
        o = np.argsort(pts[:, d], kind="stable")
        rec(ids[o[:half]])
        rec(ids[o[half:]])

    rec(np.arange(len(q)))
    return np.concatenate(out)


def _build_plan(q, k, hc=0.30, hm=0.15, hf=0.05, hg=0.08,
                refine_thr_m=3.0, refine_thr=0.9, safety=1.01):
    lo = float(min(q.min(), k.min())) - 1e-4
    hi = float(max(q.max(), k.max())) + 1e-4

    # per-query upper bound on the 8-NN distance: coarse everywhere, then
    # medium / fine refinement where the bound is already small
    nc_ = int(np.ceil((hi - lo) / hc))
    qic = np.clip(((q - lo) / hc).astype(np.int64), 0, nc_ - 1)
    _, cntc = _cell_counts(k, lo, hc, nc_)
    cells_u, inv = np.unique(qic, axis=0, return_inverse=True)
    offc, wdc = _sorted_offsets(nc_)
    ub = _walk_ub(cells_u, cntc, nc_, offc, wdc, hc, KSAFE)[inv]
    assert np.isfinite(ub).all()

    for h_r, thr in ((hm, refine_thr_m), (hf, refine_thr)):
        n_r = int(np.ceil((hi - lo) / h_r))
        qir = np.clip(((q - lo) / h_r).astype(np.int64), 0, n_r - 1)
        _, cnt_r = _cell_counts(k, lo, h_r, n_r)
        ref = ub <= thr
        if not ref.any():
            continue
        cells_r, invr = np.unique(qir[ref], axis=0, return_inverse=True)
        off_r, wd_r = _sorted_offsets(int(np.ceil(thr / h_r)) + 1)
        ubr = _walk_ub(cells_r, cnt_r, n_r, off_r, wd_r, h_r, KSAFE)[invr]
        idx = np.nonzero(ref)[0]
        better = ubr < ub[ref]
        ub[idx[better]] = ubr[better]
    ub *= safety

    perm = _kd_tiles(q)

    # gather CSR over the gather grid
    ng = int(np.ceil((hi - lo) / hg))
    kig = np.clip(((k - lo) / hg).astype(np.int64), 0, ng - 1)
    kcell = (kig[:, 0] * ng + kig[:, 1]) * ng + kig[:, 2]
    korder = np.argsort(kcell, kind="stable")
    kcs = kcell[korder]
    starts = np.searchsorted(kcs, np.arange(ng * ng * ng))
    ends = np.searchsorted(kcs, np.arange(ng * ng * ng), side="right")

    def gather(qs):
        R = float(ub[qs].max())
        R2 = R * R
        blo, bhi = q[qs].min(0), q[qs].max(0)
        a = np.maximum(((blo - R - lo) / hg).astype(np.int64), 0)
        b = np.minimum(((bhi + R - lo) / hg).astype(np.int64), ng - 1)
        parts = []
        for ix in range(a[0], b[0] + 1):
            cx0, cx1 = lo + ix * hg, lo + (ix + 1) * hg
            dx = max(blo[0] - cx1, cx0 - bhi[0], 0.0)
            if dx * dx > R2:
                continue
            for iy in range(a[1], b[1] + 1):
                cy0, cy1 = lo + iy * hg, lo + (iy + 1) * hg
                dy = max(blo[1] - cy1, cy0 - bhi[1], 0.0)
                dxy2 = dx * dx + dy * dy
                if dxy2 > R2:
                    continue
                zh = float(np.sqrt(R2 - dxy2))
                z0 = max(int((blo[2] - zh - lo) / hg), 0)
                z1 = min(int((bhi[2] + zh - lo) / hg), ng - 1)
                base = (ix * ng + iy) * ng
                s, e = starts[base + z0], ends[base + z1]
                if e > s:
                    parts.append(korder[s:e])
        if not parts:
            return np.empty(0, np.int64)
        cand = np.concatenate(parts)
        # exact filter: keep targets within R of the tile bbox
        kc = k[cand]
        dd = np.maximum(np.maximum(blo - kc, kc - bhi), 0.0)
        return cand[(dd * dd).sum(1) <= R2]

    # adaptive tiles: start from 128-query kd leaves; split a tile while the
    # two halves' candidate sets are sufficiently smaller than the parent's
    tiles = []

    def consider(qs, cand, depth):
        if len(cand) > 1024 and len(qs) >= 64 and depth < 4:
            pts = q[qs]
            dim = np.argmax(pts.max(0) - pts.min(0))
            o = np.argsort(pts[:, dim], kind="stable")
            half = len(qs) // 2
            qa, qb = qs[o[:half]], qs[o[half:]]
            ca, cb = gather(qa), gather(qb)
            if len(ca) + len(cb) + 384 < len(cand):
                consider(qa, ca, depth + 1)
                consider(qb, cb, depth + 1)
                return
        tiles.append((qs, cand))

    for t in range(NTILES):
        qs = perm[t * TILE:(t + 1) * TILE]
        consider(qs, gather(qs), 0)

    # pad tile count to a multiple of N_CORES with empty dummy tiles
    while len(tiles) % N_CORES != 0:
        tiles.append((np.empty(0, np.int64), np.empty(0, np.int64)))

    # deal tiles to cores/slots: sort by size asc (small slots first for fast
    # pipeline start), slot i <- tiles [8i, 8i+8)
    sizes = np.array([len(c) for _, c in tiles])
    order = np.argsort(sizes, kind="stable")
    rt = len(tiles) // N_CORES
    tile_of = order.reshape(rt, N_CORES)            # [slot, core]
    caps = np.empty(rt, np.int64)
    for i in range(rt):
        caps[i] = max(int(np.ceil(sizes[tile_of[i]].max() / 32.0)) * 32, 32)
    return tiles, tile_of, caps


def _f16_split(x):
    h = x.astype(np.float16)
    l = (x - h.astype(np.float32)).astype(np.float16)
    return h, l


def _rhs_block(kpts):
    """[11, C] fp16 block: rows = [(2k)h x3, (2k)h x3, (2k)l x3, -|k|2h, -|k|2l]."""
    k2 = (kpts * kpts).sum(1, dtype=np.float32)
    kh, kl = _f16_split(2.0 * kpts.T)
    k2h, k2l = _f16_split(k2)
    blk = np.empty((KDIM, len(kpts)), np.float16)
    blk[0:3] = kh
    blk[3:6] = kh
    blk[6:9] = kl
    blk[9] = -k2h
    blk[10] = -k2l
    return blk


def _layout(caps):
    """Shared DRAM layout plan for rhs_all.

    Small slots (cap <= 512) are grouped into batches of <= BATCH_MAX columns;
    within a batch, slot j rides PE row-group 32*(j%4) and the DRAM block
    orders slots by row-group so each row-group is one contiguous DMA.
    Big slots follow, laid out contiguously per slot.
    Returns (batches, bigs, slot_off, capsum) where batches is a list of
    [(slot, rg, dram_off, sbuf_off)], bigs a list of slot ids.
    """
    rt = len(caps)
    batches = []
    off = 0
    slot_off = [0] * rt
    i = 0
    while i < rt and caps[i] <= 512:
        j = i
        tot = 0
        bmax = 1024 if not batches else BATCH_MAX
        while j < rt and caps[j] <= 512 and tot + caps[j] <= bmax:
            tot += caps[j]
            j += 1
        slots = list(range(i, j))
        rg_of = {s: idx % 4 for idx, s in enumerate(slots)}
        entries = []
        sbuf_off = {}
        # sbuf offsets restart per row-group
        for p in range(4):
            sb = 0
            for s in slots:
                if rg_of[s] == p:
                    sbuf_off[s] = sb
                    sb += caps[s]
        for p in range(4):
            for s in slots:
                if rg_of[s] == p:
                    entries.append((s, p, off, sbuf_off[s]))
                    slot_off[s] = off
                    off += caps[s]
        batches.append(entries)
        i = j
    bigs = list(range(i, rt))
    for s in bigs:
        slot_off[s] = off
        off += caps[s]
    return batches, bigs, slot_off, off


def _prep_pruned(to_filter, target_coords):
    q = np.ascontiguousarray(np.asarray(to_filter, np.float32)[:, :3])
    k = np.ascontiguousarray(np.asarray(target_coords, np.float32)[:, :3])
    tiles, tile_of, caps = _build_plan(q, k)
    rt = len(caps)
    capsum = int(caps.sum())
    _, _, slot_off, lay_total = _layout(tuple(int(x) for x in caps))
    assert lay_total == capsum

    sent = np.full(3, SENTINEL, np.float32)
    in_maps = []
    rows_per_core = []
    for c in range(N_CORES):
        qsel = np.zeros((rt, TILE), np.int64)
        rows = []
        rhs_all = np.empty((KDIM, capsum), np.float16)
        for i in range(rt):
            t = tile_of[i, c]
            qs, cand = tiles[t]
            rows.append(qs)
            if len(qs):
                qsel[i, :len(qs)] = qs
                qsel[i, len(qs):] = qs[0]
            cap = int(caps[i])
            kp = np.empty((cap, 3), np.float32)
            kp[:len(cand)] = k[cand]
            kp[len(cand):] = sent
            rhs_all[:, slot_off[i]:slot_off[i] + cap] = _rhs_block(kp)
        rows_per_core.append(rows)
        qc = q[qsel.ravel()]
        q2 = (qc * qc).sum(1, dtype=np.float32)
        qh, ql = _f16_split(qc.T)
        lhsT = np.empty((KDIM, rt * TILE), np.float16)
        lhsT[0:3] = qh
        lhsT[3:6] = ql
        lhsT[6:9] = qh
        lhsT[9] = 1.0
        lhsT[10] = 1.0
        q2c = q2.reshape(rt, TILE).T                       # [128, rt]
        in_maps.append({
            "lhsT": np.ascontiguousarray(lhsT),
            "rhs_all": np.ascontiguousarray(rhs_all),
            "q2rep": np.ascontiguousarray(np.repeat(q2c, KNN, axis=1)),
        })
    return in_maps, rows_per_core, tuple(int(x) for x in caps)


def _build_pruned(caps):
    key = ("pruned", caps)
    if key in _CACHE:
        return _CACHE[key]
    from concourse import bacc, tile, mybir

    dt = mybir.dt
    capsum = sum(caps)
    rt = len(caps)
    npc = rt * TILE
    nc = bacc.Bacc("TRN2", target_bir_lowering=False, debug=False,
                   num_devices=N_CORES)

    lhsT_d = nc.dram_tensor("lhsT", [KDIM, npc], dt.float16, kind="ExternalInput")
    rhs_d = nc.dram_tensor("rhs_all", [KDIM, capsum], dt.float16,
                           kind="ExternalInput")
    q2_d = nc.dram_tensor("q2rep", [128, rt * KNN], dt.float32,
                          kind="ExternalInput")
    out_d = nc.dram_tensor("out", [128, rt * KNN], dt.float32,
                           kind="ExternalOutput")

    with tile.TileContext(nc) as tc:
        with (
            tc.tile_pool(name="const", bufs=1) as constp,
            tc.tile_pool(name="rhs", bufs=6) as rhsp,
            tc.tile_pool(name="psum", bufs=2, space="PSUM") as psump,
            tc.tile_pool(name="cand", bufs=2) as candp,
            tc.tile_pool(name="fin", bufs=1) as finp,
        ):
            # lhs split: a small first piece lets slot 0's matmul start while
            # the rest of the queries stream in; 4 replicas, one per PE
            # row-group
            lhs_sb = constp.tile([128, npc], dt.float16)
            l0 = min(8 * TILE, npc)
            for p in range(4):
                eng = nc.sync if p % 2 == 0 else nc.scalar
                eng.dma_start(out=lhs_sb[32 * p:32 * p + KDIM, :l0],
                              in_=lhsT_d[:, :l0])

            # q2 rides the SWDGE path so it never queues behind the rhs
            # stream on either HWDGE ring
            q2_sb = constp.tile([128, rt * KNN], dt.float32)
            nc.gpsimd.dma_start(out=q2_sb[:, :], in_=q2_d[:, :])
            s8_all = finp.tile([128, rt * KNN], dt.float32)
            dsq = finp.tile([128, rt * KNN], dt.float32)
            droot = finp.tile([128, rt * KNN], dt.float32)
            good = finp.tile([128, rt], dt.float32)
            res = finp.tile([128, rt * KNN], dt.float32)

            def epilogue(a, b):
                # d = sqrt(max(q2 - s, 0)); zero rows whose min d2 <= OCC^2
                ca, cb = a * KNN, b * KNN
                nc.vector.tensor_sub(dsq[:, ca:cb], q2_sb[:, ca:cb],
                                     s8_all[:, ca:cb])
                nc.vector.tensor_scalar(good[:, a:b], dsq[:, ca:cb:KNN],
                                        OCC_RADIUS * OCC_RADIUS, None,
                                        mybir.AluOpType.is_gt)
                nc.vector.tensor_scalar_max(dsq[:, ca:cb], dsq[:, ca:cb], 0.0)
                nc.scalar.activation(droot[:, ca:cb], dsq[:, ca:cb],
                                     mybir.ActivationFunctionType.Sqrt)
                nc.vector.tensor_tensor(
                    res[:, ca:cb].rearrange("p (t j) -> p t j", j=KNN),
                    droot[:, ca:cb].rearrange("p (t j) -> p t j", j=KNN),
                    good[:, a:b, None].broadcast_to([128, b - a, KNN]),
                    mybir.AluOpType.mult,
                )
                nc.sync.dma_start(out=out_d.ap()[:, ca:cb],
                                  in_=res[:, ca:cb])

            batches, bigs, slot_off, _ = _layout(caps)

            # Small slots (cap <= 512): 4 consecutive slots ride the 4 PE
            # row-groups concurrently; one DMA per row-group per batch.
            nbatch = 0
            lhs_rest_sent = False
            for entries in batches:
                rhs_sb = rhsp.tile([128, BATCH_MAX], dt.float16, tag="rhsb")
                for p in range(4):
                    rg = [e for e in entries if e[1] == p]
                    if not rg:
                        continue
                    d0 = rg[0][2]
                    tot = sum(caps[e[0]] for e in rg)
                    eng = nc.sync if (nbatch + p) % 2 == 0 else nc.scalar
                    nc_eng = eng
                    nc_eng.dma_start(
                        out=rhs_sb[32 * p:32 * p + KDIM, :tot],
                        in_=rhs_d[:, d0:d0 + tot])
                nbatch += 1
                if nbatch == 2 and not lhs_rest_sent and l0 < npc:
                    for p in range(4):
                        eng = nc.sync if p % 2 == 0 else nc.scalar
                        eng.dma_start(out=lhs_sb[32 * p:32 * p + KDIM, l0:],
                                      in_=lhsT_d[:, l0:])
                    lhs_rest_sent = True
                for s, p, d0, sb0 in entries:
                    cap = caps[s]
                    tcol = slice(s * TILE, (s + 1) * TILE)
                    ps = psump.tile([128, CHUNK], dt.float32, tag="pss",
                                    bufs=4)
                    nc.tensor.matmul(
                        out=ps[:, :cap],
                        lhsT=lhs_sb[32 * p:32 * p + KDIM, tcol],
                        rhs=rhs_sb[32 * p:32 * p + KDIM, sb0:sb0 + cap],
                        start=True, stop=True,
                        tile_position=(32 * p, 0),
                    )
                    nc.vector.max(out=s8_all[:, s * KNN:(s + 1) * KNN],
                                  in_=ps[:, :cap])
            if not lhs_rest_sent and l0 < npc:
                for p in range(4):
                    eng = nc.sync if p % 2 == 0 else nc.scalar
                    eng.dma_start(out=lhs_sb[32 * p:32 * p + KDIM, l0:],
                                  in_=lhsT_d[:, l0:])
                lhs_rest_sent = True
            epi_done = bigs[0] if bigs else rt
            epilogue(0, epi_done)

            # Big slots: 1024-col PSUM groups, the 2 chunks of a group ride
            # PE row-groups 0/32, one HWDGE ring per chunk.
            for s in bigs:
                cap = caps[s]
                ngroups = (cap + GROUP - 1) // GROUP
                cands = None
                if ngroups > 1:
                    cands = candp.tile([128, ngroups * KNN], dt.float32,
                                       tag="cands")
                tcol = slice(s * TILE, (s + 1) * TILE)
                for g in range(ngroups):
                    g0 = g * GROUP
                    gw = min(GROUP, cap - g0)
                    widths = [min(CHUNK, gw - j * CHUNK)
                              for j in range((gw + CHUNK - 1) // CHUNK)]
                    rhs_sb = rhsp.tile([64, GROUP], dt.float16, tag="rhs")
                    base = slot_off[s] + g0
                    c0 = 0
                    for j, w in enumerate(widths):
                        p = 32 * (j % 2)
                        eng = nc.sync if j % 2 == 0 else nc.scalar
                        eng.dma_start(out=rhs_sb[p:p + KDIM, :w],
                                      in_=rhs_d[:, base + c0:base + c0 + w])
                        c0 += w
                    ps = psump.tile([128, GROUP], dt.float32, tag="ps",
                                    bufs=2)
                    c0 = 0
                    for j, w in enumerate(widths):
                        p = 32 * (j % 2)
                        nc.tensor.matmul(
                            out=ps[:, c0:c0 + w],
                            lhsT=lhs_sb[p:p + KDIM, tcol],
                            rhs=rhs_sb[p:p + KDIM, :w],
                            start=True, stop=True,
                            tile_position=(p, 0),
                        )
                        c0 += w
                    dst = (s8_all[:, s * KNN:(s + 1) * KNN] if ngroups == 1
                           else cands[:, g * KNN:(g + 1) * KNN])
                    nc.vector.max(out=dst, in_=ps[:, :gw])
                if ngroups > 1:
                    nc.vector.max(out=s8_all[:, s * KNN:(s + 1) * KNN],
                                  in_=cands[:, :])
            if epi_done < rt:
                epilogue(epi_done, rt)

    nc.compile()
    _CACHE[key] = nc
    return nc


def _run(to_filter, target_coords, trace=False):
    from concourse import bass_utils

    in_maps, rows_per_core, caps = _prep_pruned(to_filter, target_coords)
    nc = _build_pruned(caps)
    res = bass_utils.run_bass_kernel_spmd(
        nc, in_maps, core_ids=list(range(N_CORES)), trace=trace,
    )
    rt = len(caps)
    out = np.empty((N, KNN), np.float32)
    for c in range(N_CORES):
        oc = res.results[c]["out"].reshape(128, rt, KNN)
        for i, qs in enumerate(rows_per_core[c]):
            if len(qs):
                out[qs] = oc[:len(qs), i, :]
    return out, res


def kernel(to_filter, target_coords):
    out, _ = _run(to_filter, target_coords)
    return out


# revision 63
# speedup vs baseline: 1.1610x; 1.0616x over previous
"""Trainium2 Bass kernel for GuidedImplicitPointSampler KNN (top-8 + occupancy mask).

Strategy (pruned, exact):
  - Host groups the N=32768 queries into 256 spatial tiles of 128 (k-d median
    splits) and, per tile, builds a provably sufficient candidate subset of the
    M=16384 targets from grid cell COUNTS only (no host distance math):
      * ub8(q): walk cell offsets sorted by worst-case point-to-point distance
        until >= 9 targets are guaranteed; d8(q) <= ub8(q).  Three-level grid
        (coarse 0.30 everywhere, 0.15 / 0.05 refines where the bound allows).
      * tile candidates: every target within R_t = max_q ub8(q) of the tile's
        bbox (cylinder-trimmed cell ranges + exact point-to-bbox filter;
        superset by construction).  Oversized tiles split adaptively.
    The device then computes exact distances + top-8 over the candidates, so
    the result equals brute force (candidates contain each query's true 8-NN
    and its nearest target, which also decides the 0.25 occupancy mask).
  - Tiles are dealt to 8 cores x rt slots (sorted by size, groups of 8) so the
    SPMD program sees identical slot capacities; blocks are sentinel-padded.
  - Per slot: s[n,m] = 2q.k - |k|^2 on the PE as one K=11 fp16 hi/lo matmul
    (error ~2^-22).  Small slots (<=512 cols) ride the 4 PE row-groups
    concurrently with batched DMAs; big slots use 1024-col PSUM groups with
    2-way row-group packing, one HWDGE ring per chunk.  Top-8 via hardware
    MAX8 straight out of PSUM (the DVE MAX8 stream is the critical path).
  - Epilogue: d = sqrt(max(q2 - s, 0)), zero rows whose nearest d2 <= 0.25^2,
    staged across the tail of the MAX8 stream; host scatters rows back to the
    original query order.
"""

import numpy as np

N = 32768
M = 16384
KNN = 8
OCC_RADIUS = 0.25
N_CORES = 8
TILE = 128
NTILES = N // TILE            # 256
RT = NTILES // N_CORES        # 32 slots per core
CHUNK = 512                   # matmul moving free dim (one PSUM bank)
GROUP = 1024                  # target cols per big-slot PSUM tile (2 chunks)
BATCH_MAX = 4096              # small-slot DMA batch columns
KDIM = 11
KSAFE = 8
SENTINEL = 60.0

_CACHE = {}


# ---------------------------------------------------------------------------
# Host-side pruning plan (grid counting only, no host distance computations)
# ---------------------------------------------------------------------------

def _cell_counts(pts, lo, h, n):
    ci = np.clip(((pts - lo) / h).astype(np.int64), 0, n - 1)
    cnt = np.zeros((n, n, n), np.int32)
    np.add.at(cnt, (ci[:, 0], ci[:, 1], ci[:, 2]), 1)
    return ci, cnt


def _sorted_offsets(max_cells):
    r = np.arange(-max_cells, max_cells + 1)
    X, Y, Z = np.meshgrid(r, r, r, indexing="ij")
    off = np.stack([X.ravel(), Y.ravel(), Z.ravel()], 1)
    wd = np.sqrt(((np.abs(off) + 1) ** 2).sum(1).astype(np.float64))
    o = np.argsort(wd, kind="stable")
    return off[o], wd[o]


def _walk_ub(cells, cnt, n, offsets, wdist, h, ksafe, chunk=512):
    """Per cell row: smallest wdist*h whose offset-prefix covers >= ksafe targets."""
    U = len(cells)
    ub = np.full(U, np.inf)
    acc = np.zeros(U, np.int64)
    alive = np.arange(U)
    for s in range(0, len(offsets), chunk):
        if len(alive) == 0:
            break
        offs = offsets[s:s + chunk]
        cc = cells[alive][:, None, :] + offs[None, :, :]
        ok = ((cc >= 0) & (cc < n)).all(2)
        cc = np.clip(cc, 0, n - 1)
        counts = cnt[cc[..., 0], cc[..., 1], cc[..., 2]] * ok
        ccum = counts.cumsum(1) + acc[alive][:, None]
        crossed = ccum >= ksafe
        hit = crossed.any(1)
        first = np.argmax(crossed, 1)
        hit_rows = alive[hit]
        ub[hit_rows] = wdist[s + first[hit]] * h
        acc[alive] = ccum[:, -1]
        alive = alive[~hit]
    return ub


def _kd_tiles(q, leaf=TILE):
    out = []

    def rec(ids):
        if len(ids) <= leaf:
            out.append(ids)
            return
        pts = q[ids]
        d = np.argmax(pts.max(0) - pts.min(0))
        half = ((len(ids) // 2) // leaf) * leaf
        o = np.argsort(pts[:, d], kind="stable")
        rec(ids[o[:half]])
        rec(ids[o[half:]])

    rec(np.arange(len(q)))
    return np.concatenate(out)


def _build_plan(q, k, hc=0.30, hm=0.15, hf=0.05, hg=0.08,
                refine_thr_m=3.0, refine_thr=0.9, safety=1.01):
    lo = float(min(q.min(), k.min())) - 1e-4
    hi = float(max(q.max(), k.max())) + 1e-4

    # per-query upper bound on the 8-NN distance: coarse everywhere, then
    # medium / fine refinement where the bound is already small
    nc_ = int(np.ceil((hi - lo) / hc))
    qic = np.clip(((q - lo) / hc).astype(np.int64), 0, nc_ - 1)
    _, cntc = _cell_counts(k, lo, hc, nc_)
    cells_u, inv = np.unique(qic, axis=0, return_inverse=True)
    offc, wdc = _sorted_offsets(nc_)
    ub = _walk_ub(cells_u, cntc, nc_, offc, wdc, hc, KSAFE)[inv]
    assert np.isfinite(ub).all()

    for h_r, thr in ((hm, refine_thr_m), (hf, refine_thr)):
        n_r = int(np.ceil((hi - lo) / h_r))
        qir = np.clip(((q - lo) / h_r).astype(np.int64), 0, n_r - 1)
        _, cnt_r = _cell_counts(k, lo, h_r, n_r)
        ref = ub <= thr
        if not ref.any():
            continue
        cells_r, invr = np.unique(qir[ref], axis=0, return_inverse=True)
        off_r, wd_r = _sorted_offsets(int(np.ceil(thr / h_r)) + 1)
        ubr = _walk_ub(cells_r, cnt_r, n_r, off_r, wd_r, h_r, KSAFE)[invr]
        idx = np.nonzero(ref)[0]
        better = ubr < ub[ref]
        ub[idx[better]] = ubr[better]
    ub *= safety

    perm = _kd_tiles(q)

    # gather CSR over the gather grid
    ng = int(np.ceil((hi - lo) / hg))
    kig = np.clip(((k - lo) / hg).astype(np.int64), 0, ng - 1)
    kcell = (kig[:, 0] * ng + kig[:, 1]) * ng + kig[:, 2]
    korder = np.argsort(kcell, kind="stable")
    kcs = kcell[korder]
    starts = np.searchsorted(kcs, np.arange(ng * ng * ng))
    ends = np.searchsorted(kcs, np.arange(ng * ng * ng), side="right")

    def gather(qs):
        R = float(ub[qs].max())
        R2 = R * R
        blo, bhi = q[qs].min(0), q[qs].max(0)
        a = np.maximum(((blo - R - lo) / hg).astype(np.int64), 0)
        b = np.minimum(((bhi + R - lo) / hg).astype(np.int64), ng - 1)
        parts = []
        for ix in range(a[0], b[0] + 1):
            cx0, cx1 = lo + ix * hg, lo + (ix + 1) * hg
            dx = max(blo[0] - cx1, cx0 - bhi[0], 0.0)
            if dx * dx > R2:
                continue
            for iy in range(a[1], b[1] + 1):
                cy0, cy1 = lo + iy * hg, lo + (iy + 1) * hg
                dy = max(blo[1] - cy1, cy0 - bhi[1], 0.0)
                dxy2 = dx * dx + dy * dy
                if dxy2 > R2:
                    continue
                zh = float(np.sqrt(R2 - dxy2))
                z0 = max(int((blo[2] - zh - lo) / hg), 0)
                z1 = min(int((bhi[2] + zh - lo) / hg), ng - 1)
                base = (ix * ng + iy) * ng
                s, e = starts[base + z0], ends[base + z1]
                if e > s:
                    parts.append(korder[s:e])
        if not parts:
            return np.empty(0, np.int64)
        cand = np.concatenate(parts)
        # exact filter: keep targets within R of the tile bbox
        kc = k[cand]
        dd = np.maximum(np.maximum(blo - kc, kc - bhi), 0.0)
        return cand[(dd * dd).sum(1) <= R2]

    # adaptive tiles: start from 128-query kd leaves; split a tile while the
    # two halves' candidate sets are sufficiently smaller than the parent's
    tiles = []

    def consider(qs, cand, depth):
        if len(cand) > 1024 and len(qs) >= 64 and depth < 5:
            pts = q[qs]
            dim = np.argmax(pts.max(0) - pts.min(0))
            o = np.argsort(pts[:, dim], kind="stable")
            half = len(qs) // 2
            qa, qb = qs[o[:half]], qs[o[half:]]
            ca, cb = gather(qa), gather(qb)
            # profitable split, or force-split oversized tiles (slot-cap
            # matching waste shrinks when sizes are more uniform)
            lim = len(cand) - 384 if len(cand) <= 3072 else int(len(cand) * 1.2)
            if len(ca) + len(cb) < lim:
                consider(qa, ca, depth + 1)
                consider(qb, cb, depth + 1)
                return
        tiles.append((qs, cand))

    for t in range(NTILES):
        qs = perm[t * TILE:(t + 1) * TILE]
        consider(qs, gather(qs), 0)

    # pad tile count to a multiple of N_CORES with empty dummy tiles
    while len(tiles) % N_CORES != 0:
        tiles.append((np.empty(0, np.int64), np.empty(0, np.int64)))

    # deal tiles to cores/slots: sort by size asc (small slots first for fast
    # pipeline start), slot i <- tiles [8i, 8i+8)
    sizes = np.array([len(c) for _, c in tiles])
    order = np.argsort(sizes, kind="stable")
    rt = len(tiles) // N_CORES
    tile_of = order.reshape(rt, N_CORES)            # [slot, core]
    caps = np.empty(rt, np.int64)
    for i in range(rt):
        caps[i] = max(int(np.ceil(sizes[tile_of[i]].max() / 32.0)) * 32, 32)

    return tiles, tile_of, caps


def _f16_split(x):
    h = x.astype(np.float16)
    l = (x - h.astype(np.float32)).astype(np.float16)
    return h, l


def _rhs_block(kpts):
    """[11, C] fp16 block: rows = [(2k)h x3, (2k)h x3, (2k)l x3, -|k|2h, -|k|2l]."""
    k2 = (kpts * kpts).sum(1, dtype=np.float32)
    kh, kl = _f16_split(2.0 * kpts.T)
    k2h, k2l = _f16_split(k2)
    blk = np.empty((KDIM, len(kpts)), np.float16)
    blk[0:3] = kh
    blk[3:6] = kh
    blk[6:9] = kl
    blk[9] = -k2h
    blk[10] = -k2l
    return blk


def _layout(caps):
    """Shared DRAM layout plan for rhs_all.

    Runs of small slots (cap <= 512) become batches of <= BATCH_MAX columns;
    within a batch, slot j rides PE row-group 32*(j%4) and the DRAM block
    orders slots by row-group so each row-group is one contiguous DMA.
    Big slots are laid out contiguously per slot where they appear.
    Returns (items, slot_off, capsum); items is an ordered list of
    ("batch", [(slot, rg, dram_off, sbuf_off)]) / ("big", slot).
    """
    rt = len(caps)
    items = []
    off = 0
    slot_off = [0] * rt
    nbatches = 0
    i = 0
    while i < rt:
        if caps[i] > 512:
            slot_off[i] = off
            off += caps[i]
            items.append(("big", i))
            i += 1
            continue
        j = i
        tot = 0
        bmax = 1024 if nbatches == 0 else BATCH_MAX
        while j < rt and caps[j] <= 512 and tot + caps[j] <= bmax:
            tot += caps[j]
            j += 1
        nbatches += 1
        slots = list(range(i, j))
        rg_of = {s: idx % 4 for idx, s in enumerate(slots)}
        entries = []
        sbuf_off = {}
        # sbuf offsets restart per row-group
        for p in range(4):
            sb = 0
            for s in slots:
                if rg_of[s] == p:
                    sbuf_off[s] = sb
                    sb += caps[s]
        for p in range(4):
            for s in slots:
                if rg_of[s] == p:
                    entries.append((s, p, off, sbuf_off[s]))
                    slot_off[s] = off
                    off += caps[s]
        items.append(("batch", entries))
        i = j

    # lhs section layout: each PE row-group replica only holds the query
    # columns of the slots that ride it (smalls by rg; bigs shared by rg 0/1)
    lhs_col = {}
    L = [0, 0, 0, 0]
    for kind, payload in items:
        if kind == "batch":
            for s, p, _, _ in payload:
                lhs_col[s] = (p, L[p])
                L[p] += TILE
    B = 0
    for kind, payload in items:
        if kind == "big":
            lhs_col[payload] = (-1, B)
            B += TILE
    return items, slot_off, off, lhs_col, L, B


def _prep_pruned(to_filter, target_coords):
    q = np.ascontiguousarray(np.asarray(to_filter, np.float32)[:, :3])
    k = np.ascontiguousarray(np.asarray(target_coords, np.float32)[:, :3])
    tiles, tile_of, caps = _build_plan(q, k)
    rt = len(caps)
    capsum = int(caps.sum())
    _, slot_off, lay_total, lhs_col, L, B = _layout(
        tuple(int(x) for x in caps))
    assert lay_total == capsum
    # DRAM lhs column of each slot: smalls in rg sections, bigs after
    sec = np.concatenate([[0], np.cumsum(L)])
    lhs_dram_col = {}
    for s, (p, col) in lhs_col.items():
        lhs_dram_col[s] = (sec[4] + col) if p < 0 else (sec[p] + col)
    lhs_w = int(sec[4] + B)

    sent = np.full(3, SENTINEL, np.float32)
    in_maps = []
    rows_per_core = []
    for c in range(N_CORES):
        qsel = np.zeros((rt, TILE), np.int64)
        rows = []
        rhs_all = np.empty((KDIM, capsum), np.float16)
        for i in range(rt):
            t = tile_of[i, c]
            qs, cand = tiles[t]
            rows.append(qs)
            if len(qs):
                qsel[i, :len(qs)] = qs
                qsel[i, len(qs):] = qs[0]
            cap = int(caps[i])
            kp = np.empty((cap, 3), np.float32)
            kp[:len(cand)] = k[cand]
            kp[len(cand):] = sent
            rhs_all[:, slot_off[i]:slot_off[i] + cap] = _rhs_block(kp)
        rows_per_core.append(rows)
        qc = q[qsel.ravel()]
        q2 = (qc * qc).sum(1, dtype=np.float32)
        qh, ql = _f16_split(qc.T)
        slotT = np.empty((KDIM, rt * TILE), np.float16)
        slotT[0:3] = qh
        slotT[3:6] = ql
        slotT[6:9] = qh
        slotT[9] = 1.0
        slotT[10] = 1.0
        # scatter slot blocks into the sectioned lhs layout
        lhsT = np.zeros((KDIM, lhs_w), np.float16)
        for s in range(rt):
            dcol = lhs_dram_col[s]
            lhsT[:, dcol:dcol + TILE] = slotT[:, s * TILE:(s + 1) * TILE]
        q2c = q2.reshape(rt, TILE).T                       # [128, rt]
        in_maps.append({
            "lhsT": np.ascontiguousarray(lhsT),
            "rhs_all": np.ascontiguousarray(rhs_all),
            "q2rep": np.ascontiguousarray(np.repeat(q2c, KNN, axis=1)),
        })
    return in_maps, rows_per_core, tuple(int(x) for x in caps)


def _build_pruned(caps):
    key = ("pruned", caps)
    if key in _CACHE:
        return _CACHE[key]
    from concourse import bacc, tile, mybir

    dt = mybir.dt
    capsum = sum(caps)
    rt = len(caps)
    items, slot_off, _, lhs_col, L, B = _layout(caps)
    sec = [0]
    for x in L:
        sec.append(sec[-1] + x)
    lhs_w = sec[4] + B
    lhs_sb_w = max(L[0] + B, L[1] + B, L[2], L[3], TILE)
    nc = bacc.Bacc("TRN2", target_bir_lowering=False, debug=False,
                   num_devices=N_CORES)

    lhsT_d = nc.dram_tensor("lhsT", [KDIM, lhs_w], dt.float16,
                            kind="ExternalInput")
    rhs_d = nc.dram_tensor("rhs_all", [KDIM, capsum], dt.float16,
                           kind="ExternalInput")
    q2_d = nc.dram_tensor("q2rep", [128, rt * KNN], dt.float32,
                          kind="ExternalInput")
    out_d = nc.dram_tensor("out", [128, rt * KNN], dt.float32,
                           kind="ExternalOutput")

    with tile.TileContext(nc) as tc:
        with (
            tc.tile_pool(name="const", bufs=1) as constp,
            tc.tile_pool(name="rhs", bufs=14) as rhsp,
            tc.tile_pool(name="psum", bufs=2, space="PSUM") as psump,
            tc.tile_pool(name="cand", bufs=2) as candp,
            tc.tile_pool(name="fin", bufs=1) as finp,
        ):
            # sectioned lhs: replica p holds only its own small slots'
            # queries; the big slots' queries are appended to replicas 0/1
            lhs_sb = constp.tile([128, lhs_sb_w], dt.float16)
            for p in range(4):
                if L[p]:
                    eng = nc.sync if p % 2 == 0 else nc.scalar
                    eng.dma_start(out=lhs_sb[32 * p:32 * p + KDIM, :L[p]],
                                  in_=lhsT_d[:, sec[p]:sec[p] + L[p]])

            # q2 rides the SWDGE path so it never queues behind the rhs
            # stream on either HWDGE ring
            q2_sb = constp.tile([128, rt * KNN], dt.float32)
            nc.gpsimd.dma_start(out=q2_sb[:, :], in_=q2_d[:, :])
            s8_all = finp.tile([128, rt * KNN], dt.float32)
            dsq = finp.tile([128, rt * KNN], dt.float32)
            droot = finp.tile([128, rt * KNN], dt.float32)
            good = finp.tile([128, rt], dt.float32)
            res = finp.tile([128, rt * KNN], dt.float32)

            def epi_stages(a, b):
                # out = sqrt((q2 - s) * mask), mask = (q2 - s) > OCC^2.
                # mask*d2 is always >= 0, so no clamp and no post-sqrt
                # multiply.  Returned as stages so callers can interleave
                # them between MAX8 work (hides the serial latency).
                ca, cb = a * KNN, b * KNN

                def s1():
                    nc.vector.tensor_sub(dsq[:, ca:cb], q2_sb[:, ca:cb],
                                         s8_all[:, ca:cb])

                def s2():
                    nc.vector.tensor_scalar(good[:, a:b], dsq[:, ca:cb:KNN],
                                            OCC_RADIUS * OCC_RADIUS, None,
                                            mybir.AluOpType.is_gt)

                def s3():
                    nc.vector.tensor_tensor(
                        res[:, ca:cb].rearrange("p (t j) -> p t j", j=KNN),
                        dsq[:, ca:cb].rearrange("p (t j) -> p t j", j=KNN),
                        good[:, a:b, None].broadcast_to([128, b - a, KNN]),
                        mybir.AluOpType.mult,
                    )

                def s4():
                    nc.scalar.activation(droot[:, ca:cb], res[:, ca:cb],
                                         mybir.ActivationFunctionType.Sqrt)
                    nc.sync.dma_start(out=out_d.ap()[:, ca:cb],
                                      in_=droot[:, ca:cb])

                return [s1, s2, s3, s4]

            def epilogue(a, b):
                for st in epi_stages(a, b):
                    st()

            # Ordered processing: runs of small slots (4-way PE row-group
            # packing, one DMA per row-group per batch) interleaved with big
            # slots (1024-col PSUM groups, 2-way row-group packing).
            nbatch = 0
            lhs_rest_sent = False
            done_cols = 0
            item_idx = 0
            n_items = len(items)
            cut_a = cut_c = 0
            pending = []          # epilogue-A stages, 1 per item
            cstages = []          # epilogue-C stages, interleaved in tail
            for kind, payload in items:
                is_last = item_idx == n_items - 1
                if kind == "batch":
                    entries = payload
                    rhs_sb = rhsp.tile([128, BATCH_MAX], dt.float16,
                                       tag="rhsb", bufs=4)
                    for p in range(4):
                        rg = [e for e in entries if e[1] == p]
                        if not rg:
                            continue
                        d0 = rg[0][2]
                        tot = sum(caps[e[0]] for e in rg)
                        eng = nc.sync if (nbatch + p) % 2 == 0 else nc.scalar
                        eng.dma_start(out=rhs_sb[32 * p:32 * p + KDIM, :tot],
                                      in_=rhs_d[:, d0:d0 + tot])
                    nbatch += 1
                    if nbatch == 2 and not lhs_rest_sent and B:
                        # big slots' queries appended to replicas 0/1
                        nc.sync.dma_start(
                            out=lhs_sb[0:KDIM, L[0]:L[0] + B],
                            in_=lhsT_d[:, sec[4]:sec[4] + B])
                        nc.scalar.dma_start(
                            out=lhs_sb[32:32 + KDIM, L[1]:L[1] + B],
                            in_=lhsT_d[:, sec[4]:sec[4] + B])
                        lhs_rest_sent = True
                    for s, p, d0, sb0 in entries:
                        cap = caps[s]
                        lc = lhs_col[s][1]
                        tcol = slice(lc, lc + TILE)
                        ps = psump.tile([128, CHUNK], dt.float32, tag="pss",
                                        bufs=4)
                        nc.tensor.matmul(
                            out=ps[:, :cap],
                            lhsT=lhs_sb[32 * p:32 * p + KDIM, tcol],
                            rhs=rhs_sb[32 * p:32 * p + KDIM, sb0:sb0 + cap],
                            start=True, stop=True,
                            tile_position=(32 * p, 0),
                        )
                        nc.vector.max(out=s8_all[:, s * KNN:(s + 1) * KNN],
                                      in_=ps[:, :cap])
                        done_cols += cap
                        last_slot = s
                        if is_last and cstages:
                            cstages.pop(0)()
                else:
                    s = payload
                    cap = caps[s]
                    ngroups = (cap + GROUP - 1) // GROUP
                    cands = None
                    if ngroups > 1:
                        cands = candp.tile([128, ngroups * KNN], dt.float32,
                                           tag="cands")
                    if item_idx == n_items - 1 and cut_a and s > cut_a:
                        # hide the next-to-last epilogue range under the
                        # final slot's group MAX8s
                        cut_c = s
                        cstages = epi_stages(cut_a, s)
                    for g in range(ngroups):
                        g0 = g * GROUP
                        gw = min(GROUP, cap - g0)
                        widths = [min(CHUNK, gw - j * CHUNK)
                                  for j in range((gw + CHUNK - 1) // CHUNK)]
                        rhs_sb = rhsp.tile([64, GROUP], dt.float16, tag="rhs")
                        base = slot_off[s] + g0
                        c0 = 0
                        for j, w in enumerate(widths):
                            p = 32 * (j % 2)
                            eng = nc.sync if j % 2 == 0 else nc.scalar
                            eng.dma_start(out=rhs_sb[p:p + KDIM, :w],
                                          in_=rhs_d[:, base + c0:base + c0 + w])
                            c0 += w
                        ps = psump.tile([128, GROUP], dt.float32, tag="ps",
                                        bufs=2)
                        c0 = 0
                        for j, w in enumerate(widths):
                            half = j % 2
                            p = 32 * half
                            bcol = L[half] + lhs_col[s][1]
                            nc.tensor.matmul(
                                out=ps[:, c0:c0 + w],
                                lhsT=lhs_sb[p:p + KDIM, bcol:bcol + TILE],
                                rhs=rhs_sb[p:p + KDIM, :w],
                                start=True, stop=True,
                                tile_position=(p, 0),
                            )
                            c0 += w
                        dst = (s8_all[:, s * KNN:(s + 1) * KNN]
                               if ngroups == 1
                               else cands[:, g * KNN:(g + 1) * KNN])
                        nc.vector.max(out=dst, in_=ps[:, :gw])
                        if cstages:
                            cstages.pop(0)()
                    if ngroups > 1:
                        nc.vector.max(out=s8_all[:, s * KNN:(s + 1) * KNN],
                                      in_=cands[:, :])
                    done_cols += cap
                    last_slot = s
                # spread epilogue-A's ops across the last items so the serial
                # sub->mask->sqrt->mult chain hides under MAX8 work
                item_idx += 1
                if cut_a == 0 and item_idx == max(n_items - 4, 1):
                    cut_a = last_slot + 1
                    pending = epi_stages(0, cut_a)
                if pending:
                    pending.pop(0)()
            while pending:
                pending.pop(0)()
            while cstages:
                cstages.pop(0)()
            if not lhs_rest_sent and B:
                nc.sync.dma_start(out=lhs_sb[0:KDIM, L[0]:L[0] + B],
                                  in_=lhsT_d[:, sec[4]:sec[4] + B])
                nc.scalar.dma_start(out=lhs_sb[32:32 + KDIM, L[1]:L[1] + B],
                                    in_=lhsT_d[:, sec[4]:sec[4] + B])
            if max(cut_c, cut_a) < rt:
                epilogue(max(cut_c, cut_a), rt)

    nc.compile()
    _CACHE[key] = nc
    return nc


def _run(to_filter, target_coords, trace=False):
    from concourse import bass_utils

    in_maps, rows_per_core, caps = _prep_pruned(to_filter, target_coords)
    nc = _build_pruned(caps)
    res = bass_utils.run_bass_kernel_spmd(
        nc, in_maps, core_ids=list(range(N_CORES)), trace=trace,
    )
    rt = len(caps)
    out = np.empty((N, KNN), np.float32)
    for c in range(N_CORES):
        oc = res.results[c]["out"].reshape(128, rt, KNN)
        for i, qs in enumerate(rows_per_core[c]):
            if len(qs):
                out[qs] = oc[:len(qs), i, :]
    return out, res


def kernel(to_filter, target_coords):
    out, _ = _run(to_filter, target_coords)
    return out


# revision 64
# speedup vs baseline: 1.2048x; 1.0377x over previous
"""Trainium2 Bass kernel for GuidedImplicitPointSampler KNN (top-8 + occupancy mask).

Strategy (pruned, exact):
  - Host groups the N=32768 queries into 256 spatial tiles of 128 (k-d median
    splits) and, per tile, builds a provably sufficient candidate subset of the
    M=16384 targets from grid cell COUNTS only (no host distance math):
      * ub8(q): walk cell offsets sorted by worst-case point-to-point distance
        until >= 9 targets are guaranteed; d8(q) <= ub8(q).  Three-level grid
        (coarse 0.30 everywhere, 0.15 / 0.05 refines where the bound allows).
      * tile candidates: every target within R_t = max_q ub8(q) of the tile's
        bbox (cylinder-trimmed cell ranges + exact point-to-bbox filter;
        superset by construction).  Oversized tiles split adaptively.
    The device then computes exact distances + top-8 over the candidates, so
    the result equals brute force (candidates contain each query's true 8-NN
    and its nearest target, which also decides the 0.25 occupancy mask).
  - Tiles are dealt to 8 cores x rt slots (sorted by size, groups of 8) so the
    SPMD program sees identical slot capacities; blocks are sentinel-padded.
  - Per slot: s[n,m] = 2q.k - |k|^2 on the PE as one K=11 fp16 hi/lo matmul
    (error ~2^-22).  Small slots (<=512 cols) ride the 4 PE row-groups
    concurrently with batched DMAs; big slots use 1024-col PSUM groups with
    2-way row-group packing, one HWDGE ring per chunk.  Top-8 via hardware
    MAX8 straight out of PSUM (the DVE MAX8 stream is the critical path).
  - Epilogue: d = sqrt(max(q2 - s, 0)), zero rows whose nearest d2 <= 0.25^2,
    staged across the tail of the MAX8 stream; host scatters rows back to the
    original query order.
"""

import numpy as np

N = 32768
M = 16384
KNN = 8
OCC_RADIUS = 0.25
N_CORES = 8
TILE = 128
NTILES = N // TILE            # 256
RT = NTILES // N_CORES        # 32 slots per core
CHUNK = 512                   # matmul moving free dim (one PSUM bank)
GROUP = 1024                  # target cols per big-slot PSUM tile (2 chunks)
BATCH_MAX = 4096              # small-slot DMA batch columns
KDIM = 11
KSAFE = 8
SENTINEL = 60.0

_CACHE = {}


# ---------------------------------------------------------------------------
# Host-side pruning plan (grid counting only, no host distance computations)
# ---------------------------------------------------------------------------

def _cell_counts(pts, lo, h, n):
    ci = np.clip(((pts - lo) / h).astype(np.int64), 0, n - 1)
    cnt = np.zeros((n, n, n), np.int32)
    np.add.at(cnt, (ci[:, 0], ci[:, 1], ci[:, 2]), 1)
    return ci, cnt


def _sorted_offsets(max_cells):
    r = np.arange(-max_cells, max_cells + 1)
    X, Y, Z = np.meshgrid(r, r, r, indexing="ij")
    off = np.stack([X.ravel(), Y.ravel(), Z.ravel()], 1)
    wd = np.sqrt(((np.abs(off) + 1) ** 2).sum(1).astype(np.float64))
    o = np.argsort(wd, kind="stable")
    return off[o], wd[o]


def _walk_ub(cells, cnt, n, offsets, wdist, h, ksafe, chunk=512):
    """Per cell row: smallest wdist*h whose offset-prefix covers >= ksafe targets."""
    U = len(cells)
    ub = np.full(U, np.inf)
    acc = np.zeros(U, np.int64)
    alive = np.arange(U)
    for s in range(0, len(offsets), chunk):
        if len(alive) == 0:
            break
        offs = offsets[s:s + chunk]
        cc = cells[alive][:, None, :] + offs[None, :, :]
        ok = ((cc >= 0) & (cc < n)).all(2)
        cc = np.clip(cc, 0, n - 1)
        counts = cnt[cc[..., 0], cc[..., 1], cc[..., 2]] * ok
        ccum = counts.cumsum(1) + acc[alive][:, None]
        crossed = ccum >= ksafe
        hit = crossed.any(1)
        first = np.argmax(crossed, 1)
        hit_rows = alive[hit]
        ub[hit_rows] = wdist[s + first[hit]] * h
        acc[alive] = ccum[:, -1]
        alive = alive[~hit]
    return ub


def _kd_tiles(q, leaf=TILE):
    out = []

    def rec(ids):
        if len(ids) <= leaf:
            out.append(ids)
            return
        pts = q[ids]
        d = np.argmax(pts.max(0) - pts.min(0))
        half = ((len(ids) // 2) // leaf) * leaf
        o = np.argsort(pts[:, d], kind="stable")
        rec(ids[o[:half]])
        rec(ids[o[half:]])

    rec(np.arange(len(q)))
    return np.concatenate(out)


def _build_plan(q, k, hc=0.30, hm=0.12, hf=0.045, hg=0.08,
                refine_thr_m=3.0, refine_thr=0.9, safety=1.01):
    lo = float(min(q.min(), k.min())) - 1e-4
    hi = float(max(q.max(), k.max())) + 1e-4

    # per-query upper bound on the 8-NN distance: coarse everywhere, then
    # medium / fine refinement where the bound is already small
    nc_ = int(np.ceil((hi - lo) / hc))
    qic = np.clip(((q - lo) / hc).astype(np.int64), 0, nc_ - 1)
    _, cntc = _cell_counts(k, lo, hc, nc_)
    cells_u, inv = np.unique(qic, axis=0, return_inverse=True)
    offc, wdc = _sorted_offsets(nc_)
    ub = _walk_ub(cells_u, cntc, nc_, offc, wdc, hc, KSAFE)[inv]
    assert np.isfinite(ub).all()

    for h_r, thr in ((hm, refine_thr_m), (hf, refine_thr)):
        n_r = int(np.ceil((hi - lo) / h_r))
        qir = np.clip(((q - lo) / h_r).astype(np.int64), 0, n_r - 1)
        _, cnt_r = _cell_counts(k, lo, h_r, n_r)
        ref = ub <= thr
        if not ref.any():
            continue
        cells_r, invr = np.unique(qir[ref], axis=0, return_inverse=True)
        off_r, wd_r = _sorted_offsets(int(np.ceil(thr / h_r)) + 1)
        ubr = _walk_ub(cells_r, cnt_r, n_r, off_r, wd_r, h_r, KSAFE)[invr]
        idx = np.nonzero(ref)[0]
        better = ubr < ub[ref]
        ub[idx[better]] = ubr[better]
    ub *= safety

    perm = _kd_tiles(q)

    # gather CSR over the gather grid
    ng = int(np.ceil((hi - lo) / hg))
    kig = np.clip(((k - lo) / hg).astype(np.int64), 0, ng - 1)
    kcell = (kig[:, 0] * ng + kig[:, 1]) * ng + kig[:, 2]
    korder = np.argsort(kcell, kind="stable")
    kcs = kcell[korder]
    starts = np.searchsorted(kcs, np.arange(ng * ng * ng))
    ends = np.searchsorted(kcs, np.arange(ng * ng * ng), side="right")

    def gather(qs):
        R = float(ub[qs].max())
        R2 = R * R
        blo, bhi = q[qs].min(0), q[qs].max(0)
        a = np.maximum(((blo - R - lo) / hg).astype(np.int64), 0)
        b = np.minimum(((bhi + R - lo) / hg).astype(np.int64), ng - 1)
        parts = []
        for ix in range(a[0], b[0] + 1):
            cx0, cx1 = lo + ix * hg, lo + (ix + 1) * hg
            dx = max(blo[0] - cx1, cx0 - bhi[0], 0.0)
            if dx * dx > R2:
                continue
            for iy in range(a[1], b[1] + 1):
                cy0, cy1 = lo + iy * hg, lo + (iy + 1) * hg
                dy = max(blo[1] - cy1, cy0 - bhi[1], 0.0)
                dxy2 = dx * dx + dy * dy
                if dxy2 > R2:
                    continue
                zh = float(np.sqrt(R2 - dxy2))
                z0 = max(int((blo[2] - zh - lo) / hg), 0)
                z1 = min(int((bhi[2] + zh - lo) / hg), ng - 1)
                base = (ix * ng + iy) * ng
                s, e = starts[base + z0], ends[base + z1]
                if e > s:
                    parts.append(korder[s:e])
        if not parts:
            return np.empty(0, np.int64)
        cand = np.concatenate(parts)
        # exact filter: keep targets within R of the tile bbox
        kc = k[cand]
        dd = np.maximum(np.maximum(blo - kc, kc - bhi), 0.0)
        return cand[(dd * dd).sum(1) <= R2]

    # adaptive tiles: start from 128-query kd leaves; split a tile while the
    # two halves' candidate sets are sufficiently smaller than the parent's
    tiles = []

    def consider(qs, cand, depth):
        if len(cand) > 1024 and len(qs) >= 64 and depth < 5:
            pts = q[qs]
            dim = np.argmax(pts.max(0) - pts.min(0))
            o = np.argsort(pts[:, dim], kind="stable")
            half = len(qs) // 2
            qa, qb = qs[o[:half]], qs[o[half:]]
            ca, cb = gather(qa), gather(qb)
            # profitable split, or force-split oversized tiles (slot-cap
            # matching waste shrinks when sizes are more uniform)
            lim = len(cand) - 384 if len(cand) <= 3072 else int(len(cand) * 1.2)
            if len(ca) + len(cb) < lim:
                consider(qa, ca, depth + 1)
                consider(qb, cb, depth + 1)
                return
        tiles.append((qs, cand))

    for t in range(NTILES):
        qs = perm[t * TILE:(t + 1) * TILE]
        consider(qs, gather(qs), 0)

    # pad tile count to a multiple of N_CORES with empty dummy tiles
    while len(tiles) % N_CORES != 0:
        tiles.append((np.empty(0, np.int64), np.empty(0, np.int64)))

    # deal tiles to cores/slots: sort by size asc (small slots first for fast
    # pipeline start), slot i <- tiles [8i, 8i+8)
    sizes = np.array([len(c) for _, c in tiles])
    order = np.argsort(sizes, kind="stable")
    rt = len(tiles) // N_CORES
    tile_of = order.reshape(rt, N_CORES)            # [slot, core]
    caps = np.empty(rt, np.int64)
    for i in range(rt):
        caps[i] = max(int(np.ceil(sizes[tile_of[i]].max() / 32.0)) * 32, 32)

    return tiles, tile_of, caps


def _f16_split(x):
    h = x.astype(np.float16)
    l = (x - h.astype(np.float32)).astype(np.float16)
    return h, l


def _rhs_block(kpts):
    """[11, C] fp16 block: rows = [(2k)h x3, (2k)h x3, (2k)l x3, -|k|2h, -|k|2l]."""
    k2 = (kpts * kpts).sum(1, dtype=np.float32)
    kh, kl = _f16_split(2.0 * kpts.T)
    k2h, k2l = _f16_split(k2)
    blk = np.empty((KDIM, len(kpts)), np.float16)
    blk[0:3] = kh
    blk[3:6] = kh
    blk[6:9] = kl
    blk[9] = -k2h
    blk[10] = -k2l
    return blk


def _layout(caps):
    """Shared DRAM layout plan for rhs_all.

    Runs of small slots (cap <= 512) become batches of <= BATCH_MAX columns;
    within a batch, slot j rides PE row-group 32*(j%4) and the DRAM block
    orders slots by row-group so each row-group is one contiguous DMA.
    Big slots are laid out contiguously per slot where they appear.
    Returns (items, slot_off, capsum); items is an ordered list of
    ("batch", [(slot, rg, dram_off, sbuf_off)]) / ("big", slot).
    """
    rt = len(caps)
    items = []
    off = 0
    slot_off = [0] * rt
    nbatches = 0
    i = 0
    while i < rt:
        if caps[i] > 512:
            slot_off[i] = off
            off += caps[i]
            items.append(("big", i))
            i += 1
            continue
        j = i
        tot = 0
        bmax = 1024 if nbatches == 0 else BATCH_MAX
        while j < rt and caps[j] <= 512 and tot + caps[j] <= bmax:
            tot += caps[j]
            j += 1
        nbatches += 1
        slots = list(range(i, j))
        rg_of = {s: idx % 4 for idx, s in enumerate(slots)}
        entries = []
        sbuf_off = {}
        # sbuf offsets restart per row-group
        for p in range(4):
            sb = 0
            for s in slots:
                if rg_of[s] == p:
                    sbuf_off[s] = sb
                    sb += caps[s]
        for p in range(4):
            for s in slots:
                if rg_of[s] == p:
                    entries.append((s, p, off, sbuf_off[s]))
                    slot_off[s] = off
                    off += caps[s]
        items.append(("batch", entries))
        i = j

    # lhs section layout: each PE row-group replica only holds the query
    # columns of the slots that ride it (smalls by rg; bigs shared by rg 0/1)
    lhs_col = {}
    L = [0, 0, 0, 0]
    for kind, payload in items:
        if kind == "batch":
            for s, p, _, _ in payload:
                lhs_col[s] = (p, L[p])
                L[p] += TILE
    B = 0
    for kind, payload in items:
        if kind == "big":
            lhs_col[payload] = (-1, B)
            B += TILE
    return items, slot_off, off, lhs_col, L, B


def _prep_pruned(to_filter, target_coords):
    q = np.ascontiguousarray(np.asarray(to_filter, np.float32)[:, :3])
    k = np.ascontiguousarray(np.asarray(target_coords, np.float32)[:, :3])
    tiles, tile_of, caps = _build_plan(q, k)
    rt = len(caps)
    capsum = int(caps.sum())
    _, slot_off, lay_total, lhs_col, L, B = _layout(
        tuple(int(x) for x in caps))
    assert lay_total == capsum
    # DRAM lhs column of each slot: smalls in rg sections, bigs after
    sec = np.concatenate([[0], np.cumsum(L)])
    lhs_dram_col = {}
    for s, (p, col) in lhs_col.items():
        lhs_dram_col[s] = (sec[4] + col) if p < 0 else (sec[p] + col)
    lhs_w = int(sec[4] + B)

    sent = np.full(3, SENTINEL, np.float32)
    in_maps = []
    rows_per_core = []
    for c in range(N_CORES):
        qsel = np.zeros((rt, TILE), np.int64)
        rows = []
        rhs_all = np.empty((KDIM, capsum), np.float16)
        for i in range(rt):
            t = tile_of[i, c]
            qs, cand = tiles[t]
            rows.append(qs)
            if len(qs):
                qsel[i, :len(qs)] = qs
                qsel[i, len(qs):] = qs[0]
            cap = int(caps[i])
            kp = np.empty((cap, 3), np.float32)
            kp[:len(cand)] = k[cand]
            kp[len(cand):] = sent
            rhs_all[:, slot_off[i]:slot_off[i] + cap] = _rhs_block(kp)
        rows_per_core.append(rows)
        qc = q[qsel.ravel()]
        q2 = (qc * qc).sum(1, dtype=np.float32)
        qh, ql = _f16_split(qc.T)
        slotT = np.empty((KDIM, rt * TILE), np.float16)
        slotT[0:3] = qh
        slotT[3:6] = ql
        slotT[6:9] = qh
        slotT[9] = 1.0
        slotT[10] = 1.0
        # scatter slot blocks into the sectioned lhs layout
        lhsT = np.zeros((KDIM, lhs_w), np.float16)
        for s in range(rt):
            dcol = lhs_dram_col[s]
            lhsT[:, dcol:dcol + TILE] = slotT[:, s * TILE:(s + 1) * TILE]
        q2c = q2.reshape(rt, TILE).T                       # [128, rt]
        in_maps.append({
            "lhsT": np.ascontiguousarray(lhsT),
            "rhs_all": np.ascontiguousarray(rhs_all),
            "q2rep": np.ascontiguousarray(np.repeat(q2c, KNN, axis=1)),
        })
    return in_maps, rows_per_core, tuple(int(x) for x in caps)


def _build_pruned(caps):
    key = ("pruned", caps)
    if key in _CACHE:
        return _CACHE[key]
    from concourse import bacc, tile, mybir

    dt = mybir.dt
    capsum = sum(caps)
    rt = len(caps)
    items, slot_off, _, lhs_col, L, B = _layout(caps)
    sec = [0]
    for x in L:
        sec.append(sec[-1] + x)
    lhs_w = sec[4] + B
    lhs_sb_w = max(L[0] + B, L[1] + B, L[2], L[3], TILE)
    nc = bacc.Bacc("TRN2", target_bir_lowering=False, debug=False,
                   num_devices=N_CORES)

    lhsT_d = nc.dram_tensor("lhsT", [KDIM, lhs_w], dt.float16,
                            kind="ExternalInput")
    rhs_d = nc.dram_tensor("rhs_all", [KDIM, capsum], dt.float16,
                           kind="ExternalInput")
    q2_d = nc.dram_tensor("q2rep", [128, rt * KNN], dt.float32,
                          kind="ExternalInput")
    out_d = nc.dram_tensor("out", [128, rt * KNN], dt.float32,
                           kind="ExternalOutput")

    with tile.TileContext(nc) as tc:
        with (
            tc.tile_pool(name="const", bufs=1) as constp,
            tc.tile_pool(name="rhs", bufs=14) as rhsp,
            tc.tile_pool(name="psum", bufs=2, space="PSUM") as psump,
            tc.tile_pool(name="cand", bufs=2) as candp,
            tc.tile_pool(name="fin", bufs=1) as finp,
        ):
            # sectioned lhs: replica p holds only its own small slots'
            # queries; the big slots' queries are appended to replicas 0/1
            lhs_sb = constp.tile([128, lhs_sb_w], dt.float16)
            for p in range(4):
                if L[p]:
                    eng = nc.sync if p % 2 == 0 else nc.scalar
                    eng.dma_start(out=lhs_sb[32 * p:32 * p + KDIM, :L[p]],
                                  in_=lhsT_d[:, sec[p]:sec[p] + L[p]])

            # q2 rides the SWDGE path so it never queues behind the rhs
            # stream on either HWDGE ring
            q2_sb = constp.tile([128, rt * KNN], dt.float32)
            nc.gpsimd.dma_start(out=q2_sb[:, :], in_=q2_d[:, :])
            s8_all = finp.tile([128, rt * KNN], dt.float32)
            dsq = finp.tile([128, rt * KNN], dt.float32)
            droot = finp.tile([128, rt * KNN], dt.float32)
            good = finp.tile([128, rt], dt.float32)
            res = finp.tile([128, rt * KNN], dt.float32)

            def epi_stages(a, b):
                # out = sqrt((q2 - s) * mask), mask = (q2 - s) > OCC^2.
                # mask*d2 is always >= 0, so no clamp and no post-sqrt
                # multiply.  Returned as stages so callers can interleave
                # them between MAX8 work (hides the serial latency).
                ca, cb = a * KNN, b * KNN

                def s1():
                    nc.vector.tensor_sub(dsq[:, ca:cb], q2_sb[:, ca:cb],
                                         s8_all[:, ca:cb])

                def s2():
                    nc.vector.tensor_scalar(good[:, a:b], dsq[:, ca:cb:KNN],
                                            OCC_RADIUS * OCC_RADIUS, None,
                                            mybir.AluOpType.is_gt)

                def s3():
                    nc.vector.tensor_tensor(
                        res[:, ca:cb].rearrange("p (t j) -> p t j", j=KNN),
                        dsq[:, ca:cb].rearrange("p (t j) -> p t j", j=KNN),
                        good[:, a:b, None].broadcast_to([128, b - a, KNN]),
                        mybir.AluOpType.mult,
                    )

                def s4():
                    nc.scalar.activation(droot[:, ca:cb], res[:, ca:cb],
                                         mybir.ActivationFunctionType.Sqrt)
                    nc.sync.dma_start(out=out_d.ap()[:, ca:cb],
                                      in_=droot[:, ca:cb])

                return [s1, s2, s3, s4]

            def epilogue(a, b):
                for st in epi_stages(a, b):
                    st()

            # Ordered processing: runs of small slots (4-way PE row-group
            # packing, one DMA per row-group per batch) interleaved with big
            # slots (1024-col PSUM groups, 2-way row-group packing).
            nbatch = 0
            lhs_rest_sent = False
            done_cols = 0
            item_idx = 0
            n_items = len(items)
            cut_a = cut_c = 0
            pending = []          # epilogue-A stages, 1 per item
            cstages = []          # epilogue-C stages, interleaved in tail
            for kind, payload in items:
                is_last = item_idx == n_items - 1
                if kind == "batch":
                    entries = payload
                    rhs_sb = rhsp.tile([128, BATCH_MAX], dt.float16,
                                       tag="rhsb", bufs=4)
                    for p in range(4):
                        rg = [e for e in entries if e[1] == p]
                        if not rg:
                            continue
                        d0 = rg[0][2]
                        tot = sum(caps[e[0]] for e in rg)
                        eng = nc.sync if (nbatch + p) % 2 == 0 else nc.scalar
                        eng.dma_start(out=rhs_sb[32 * p:32 * p + KDIM, :tot],
                                      in_=rhs_d[:, d0:d0 + tot])
                    nbatch += 1
                    if nbatch == 2 and not lhs_rest_sent and B:
                        # big slots' queries appended to replicas 0/1
                        nc.sync.dma_start(
                            out=lhs_sb[0:KDIM, L[0]:L[0] + B],
                            in_=lhsT_d[:, sec[4]:sec[4] + B])
                        nc.scalar.dma_start(
                            out=lhs_sb[32:32 + KDIM, L[1]:L[1] + B],
                            in_=lhsT_d[:, sec[4]:sec[4] + B])
                        lhs_rest_sent = True
                    for s, p, d0, sb0 in entries:
                        cap = caps[s]
                        lc = lhs_col[s][1]
                        tcol = slice(lc, lc + TILE)
                        ps = psump.tile([128, CHUNK], dt.float32, tag="pss",
                                        bufs=4)
                        nc.tensor.matmul(
                            out=ps[:, :cap],
                            lhsT=lhs_sb[32 * p:32 * p + KDIM, tcol],
                            rhs=rhs_sb[32 * p:32 * p + KDIM, sb0:sb0 + cap],
                            start=True, stop=True,
                            tile_position=(32 * p, 0),
                        )
                        nc.vector.max(out=s8_all[:, s * KNN:(s + 1) * KNN],
                                      in_=ps[:, :cap])
                        done_cols += cap
                        last_slot = s
                        if is_last and cstages:
                            cstages.pop(0)()
                else:
                    s = payload
                    cap = caps[s]
                    ngroups = (cap + GROUP - 1) // GROUP
                    cands = None
                    if ngroups > 1:
                        cands = candp.tile([128, ngroups * KNN], dt.float32,
                                           tag="cands")
                    if item_idx == n_items - 1 and cut_a and s > cut_a:
                        # hide the next-to-last epilogue range under the
                        # final slot's group MAX8s
                        cut_c = s
                        cstages = epi_stages(cut_a, s)
                    for g in range(ngroups):
                        g0 = g * GROUP
                        gw = min(GROUP, cap - g0)
                        widths = [min(CHUNK, gw - j * CHUNK)
                                  for j in range((gw + CHUNK - 1) // CHUNK)]
                        rhs_sb = rhsp.tile([64, GROUP], dt.float16, tag="rhs")
                        base = slot_off[s] + g0
                        c0 = 0
                        for j, w in enumerate(widths):
                            p = 32 * (j % 2)
                            eng = nc.sync if j % 2 == 0 else nc.scalar
                            eng.dma_start(out=rhs_sb[p:p + KDIM, :w],
                                          in_=rhs_d[:, base + c0:base + c0 + w])
                            c0 += w
                        ps = psump.tile([128, GROUP], dt.float32, tag="ps",
                                        bufs=2)
                        c0 = 0
                        for j, w in enumerate(widths):
                            half = j % 2
                            p = 32 * half
                            bcol = L[half] + lhs_col[s][1]
                            nc.tensor.matmul(
                                out=ps[:, c0:c0 + w],
                                lhsT=lhs_sb[p:p + KDIM, bcol:bcol + TILE],
                                rhs=rhs_sb[p:p + KDIM, :w],
                                start=True, stop=True,
                                tile_position=(p, 0),
                            )
                            c0 += w
                        dst = (s8_all[:, s * KNN:(s + 1) * KNN]
                               if ngroups == 1
                               else cands[:, g * KNN:(g + 1) * KNN])
                        nc.vector.max(out=dst, in_=ps[:, :gw])
                        if cstages:
                            cstages.pop(0)()
                    if ngroups > 1:
                        nc.vector.max(out=s8_all[:, s * KNN:(s + 1) * KNN],
                                      in_=cands[:, :])
                    done_cols += cap
                    last_slot = s
                # spread epilogue-A's ops across the last items so the serial
                # sub->mask->sqrt->mult chain hides under MAX8 work
                item_idx += 1
                if cut_a == 0 and item_idx == max(n_items - 4, 1):
                    cut_a = last_slot + 1
                    pending = epi_stages(0, cut_a)
                if pending:
                    pending.pop(0)()
            while pending:
                pending.pop(0)()
            while cstages:
                cstages.pop(0)()
            if not lhs_rest_sent and B:
                nc.sync.dma_start(out=lhs_sb[0:KDIM, L[0]:L[0] + B],
                                  in_=lhsT_d[:, sec[4]:sec[4] + B])
                nc.scalar.dma_start(out=lhs_sb[32:32 + KDIM, L[1]:L[1] + B],
                                    in_=lhsT_d[:, sec[4]:sec[4] + B])
            if max(cut_c, cut_a) < rt:
                epilogue(max(cut_c, cut_a), rt)

    nc.compile()
    _CACHE[key] = nc
    return nc


def _run(to_filter, target_coords, trace=False):
    from concourse import bass_utils

    in_maps, rows_per_core, caps = _prep_pruned(to_filter, target_coords)
    nc = _build_pruned(caps)
    res = bass_utils.run_bass_kernel_spmd(
        nc, in_maps, core_ids=list(range(N_CORES)), trace=trace,
    )
    rt = len(caps)
    out = np.empty((N, KNN), np.float32)
    for c in range(N_CORES):
        oc = res.results[c]["out"].reshape(128, rt, KNN)
        for i, qs in enumerate(rows_per_core[c]):
            if len(qs):
                out[qs] = oc[:len(qs), i, :]
    return out, res


def kernel(to_filter, target_coords):
    out, _ = _run(to_filter, target_coords)
    return out
